# revision 10
# baseline (speedup 1.0000x reference)
"""IsoMaxPlus first-part kernel for TRN2 (8 NeuronCores, data-parallel on B).

out[b, c] = -|s| * sqrt(max(2 - 2 * <f_b/||f_b||, p_c/||p_c||>, 1e-12))

Host prep (per core shard of 8192 rows): features are cast to bf16 and
pre-transposed to d-major layout [128 dpart, 64 blocks, 512] so the device
needs no DMA transposes or casts; prototypes are zero-padded to [1024, 512].

Device per core:
  prolog: 8 big DMAs load all features into SBUF (64KB/partition);
          prototypes are row-normalized (fp32, negated scale, bf16 cast)
          then transposed 128x128-wise on the TensorEngine into
          pnT [128, 4, 1024] bf16.
  main:   8 groups x 8 blocks of 128 rows. Per block: DVE squares the
          bf16 feature slab, PE reduces it against a ones-vector into
          n2 (row norms, exact fp32 psum accumulation), DVE copies n2
          out; per group one ACT Sqrt + DVE reciprocal turn 8 norms into
          the fused scale 2s^2/||f||. Then per block 8 bf16 matmuls
          accumulate dots=-f.p_hat into psum [128,1000], ACT computes
          sqrt(scale*dots + 2s^2) = |s|*dist, GpSimd negates, SP DMAs out.
All matmuls run back-to-back so the PE stays HAM-warm (2.4 GHz).
"""

import numpy as np
from contextlib import ExitStack

import ml_dtypes

import concourse.bass as bass
import concourse.tile as tile
from concourse import bacc, masks, mybir
from concourse.bass import ts
from concourse.bass_utils import run_bass_kernel_spmd

N_CORES = 8
B, D, C = 65536, 512, 1000
CP = 1024                  # prototypes padded (zeros) for 128-alignment
BS = B // N_CORES          # 8192 rows per core
NB = BS // 128             # 64 row blocks
KC = D // 128              # 4 contraction chunks
GB = 8                     # blocks per norm group
NSPLIT = (512, C - 512)    # psum halves (max moving free dim = 512)
CSPLIT = ((0, 256), (256, 256), (512, 256), (768, C - 768))  # fp8 DR chunks
F32 = mybir.dt.float32
BF16 = mybir.dt.bfloat16
F8 = mybir.dt.float8e4
SQRT = mybir.ActivationFunctionType.Sqrt
SQUARE = mybir.ActivationFunctionType.Square


def _emit(nc):
    f_dram = nc.dram_tensor("features", [128, NB, KC, 128], F8, kind="ExternalInput").ap()
    p_dram = nc.dram_tensor("prototypes", [CP, D], F32, kind="ExternalInput").ap()
    s_dram = nc.dram_tensor("distance_scale", [1], F32, kind="ExternalInput").ap()
    o_dram = nc.dram_tensor("out", [BS, C], F32, kind="ExternalOutput").ap()
    o_quad = o_dram.rearrange("(n j p) c -> p n j c", n=NB // 4, j=4, p=128)

    with tile.TileContext(nc) as tc, ExitStack() as ctx:
        singles = ctx.enter_context(tc.tile_pool(name="singles", bufs=1))

        fT = singles.tile([128, NB, KC, 128], F8)  # all features (fp8), 32KB/part
        pnT = singles.tile([128, KC, CP], BF16)    # -p_hat transposed
        identity = singles.tile([128, 128], BF16)
        ones1 = singles.tile([128, 1], BF16)
        n2a = singles.tile([128, NB], F32)         # row norms^2
        scal = singles.tile([128, NB], F32)        # 2s^2 / ||f||
        s_b = singles.tile([128, 1], F32)
        two_s2 = singles.tile([128, 1], F32)
        inv4s4 = singles.tile([128, 1], F32)

        nc.gpsimd.dma_start(out=s_b[:], in_=s_dram.to_broadcast([128, 1]))
        nc.gpsimd.memset(ones1[:], 1.0)
        masks.make_identity(nc, identity[:])
        s2t = singles.tile([128, 1], F32)
        nc.vector.tensor_mul(s2t[:], s_b[:], s_b[:])
        nc.vector.tensor_scalar_mul(two_s2[:], s2t[:], 2.0)
        q4t = singles.tile([128, 1], F32)
        nc.vector.tensor_mul(q4t[:], two_s2[:], two_s2[:])
        nc.vector.reciprocal(inv4s4[:], q4t[:])

        # First two feature groups in flight before the prototype loads so
        # group-0/1 norm phases are not DMA-gated.
        for gi in range(2):
            nc.sync.dma_start(out=fT[:, ts(gi, GB)], in_=f_dram[:, ts(gi, GB)])

        ADD = mybir.AluOpType.add
        npsum = ctx.enter_context(tc.tile_pool(name="npsum", bufs=2, space="PSUM"))
        fsqp = ctx.enter_context(tc.tile_pool(name="fsqp", bufs=2))
        fs2p = ctx.enter_context(tc.tile_pool(name="fs2p", bufs=3))
        fsump = ctx.enter_context(tc.tile_pool(name="fsump", bufs=3))
        gsml = ctx.enter_context(tc.tile_pool(name="gsml", bufs=2))
        opool = ctx.enter_context(tc.tile_pool(name="opool", bufs=4))

        def emit_norm_phase(g):
            # Row norms for a whole group (emitted one group ahead of its
            # dots phase so neither PE nor ACT hits a group-boundary stall).
            # One ACT op squares the whole group's slab (amortizes the
            # 352-cycle ACT fixed cost); PE reduces per block vs ones.
            fsq = fsqp.tile([128, GB, KC, 128], BF16, tag="fsq")
            nc.scalar.activation(fsq[:], fT[:, ts(g, GB)], SQUARE)
            for j in range(GB):
                ib = g * GB + j
                # pairwise-sum the 4 k-chunks on DVE so the PE does one
                # ones-matmul instead of four (bf16 partials: ~0.03% n2 err)
                fs2 = fs2p.tile([128, 2, 128], BF16, tag="fs2")
                nc.vector.tensor_tensor(
                    fs2[:], fsq[:, j, 0:2, :], fsq[:, j, 2:4, :], op=ADD,
                )
                fsum = fsump.tile([128, 128], BF16, tag="fsum")
                nc.vector.tensor_tensor(
                    fsum[:], fs2[:, 0, :], fs2[:, 1, :], op=ADD,
                )
                n2p = npsum.tile([128, 1], F32, tag="n2p")
                nc.tensor.matmul(n2p[:], fsum[:], ones1[:],
                                 skip_group_check=True)
                nc.vector.tensor_copy(n2a[:, ib : ib + 1], n2p[:])
            qg = gsml.tile([128, GB], F32, tag="qg")
            nc.scalar.activation(qg[:], n2a[:, ts(g, GB)], SQRT, scale=inv4s4[:])
            nc.vector.reciprocal(scal[:, ts(g, GB)], qg[:])

        # Norm phases for groups 0/1 ahead of the prototype chain: the PE
        # starts on their ones-matmuls while prototypes are still loading.
        emit_norm_phase(0)
        emit_norm_phase(1)

        # ---- prototypes: normalize rows (negated), TensorE-transpose ----
        with tc.tile_pool(name="tpsum", bufs=2, space="PSUM") as tpsum, \
             tc.tile_pool(name="ppool", bufs=2) as ppool, \
             tc.tile_pool(name="psml", bufs=2) as psml:
            for cb in range(CP // 128):
                pt = ppool.tile([128, D], F32, tag="pt")
                nc.sync.dma_start(out=pt[:], in_=p_dram[ts(cb, 128), :])
                pn2 = psml.tile([128, 1], F32, tag="pn2")
                psq = ppool.tile([128, D], F32, tag="psq")
                nc.scalar.activation(psq[:], pt[:], SQUARE, accum_out=pn2[:])
                nc.scalar.sqrt(pn2[:], pn2[:])
                nc.vector.tensor_scalar_max(pn2[:], pn2[:], 1e-12)
                npri = psml.tile([128, 1], F32, tag="npri")
                nc.vector.reciprocal(npri[:], pn2[:])
                nc.vector.tensor_scalar_mul(npri[:], npri[:], -1.0)
                pnb = ppool.tile([128, D], BF16, tag="pnb")
                nc.vector.tensor_scalar_mul(pnb[:], pt[:], npri[:])
                for kc in range(KC):
                    pst = tpsum.tile([128, 128], BF16, tag="pst")
                    nc.tensor.transpose(pst[:], pnb[:, ts(kc, 128)], identity[:])
                    nc.vector.tensor_copy(pnT[:, kc, ts(cb, 128)], pst[:])

        for gi in range(2, NB // GB):
            nc.sync.dma_start(out=fT[:, ts(gi, GB)], in_=f_dram[:, ts(gi, GB)])

        mpsum = ctx.enter_context(tc.tile_pool(name="mpsum", bufs=3, space="PSUM"))

        def emit_dots_phase(g):
            for j in range(0, GB, 4):
                ot4 = opool.tile([128, 4, C], F32, tag="ot4")
                for h in range(4):
                    ib = g * GB + j + h
                    dots = mpsum.tile([128, C], F32, tag="dots")
                    for kc in range(KC):
                        for lo, width in ((0, NSPLIT[0]), (NSPLIT[0], NSPLIT[1])):
                            nc.tensor.matmul(
                                dots[:, lo : lo + width],
                                fT[:, ib, kc, :],
                                pnT[:, kc, lo : lo + width],
                                start=(kc == 0), stop=(kc == KC - 1),
                                skip_group_check=True,
                            )
                    nc.scalar.activation(
                        ot4[:, h], dots[:], SQRT,
                        bias=two_s2[:], scale=scal[:, ib : ib + 1],
                    )
                # negate + store four blocks at a time (fewer DVE/DMA ops)
                nc.vector.tensor_scalar_mul(ot4[:], ot4[:], -1.0)
                nc.gpsimd.dma_start(out=o_quad[:, (g * GB + j) // 4], in_=ot4[:])

        for g in range(NB // GB):
            emit_dots_phase(g)
            if g + 2 < NB // GB:
                emit_norm_phase(g + 2)


def build():
    nc = bacc.Bacc("TRN2", target_bir_lowering=False, debug=False,
                   num_devices=N_CORES)
    _emit(nc)
    nc.compile()
    return nc


def _ensure_ntff_hook():
    """Dev-only: restore the axon NTFF profile hook that the trimmed agent
    image's antenv package lacks, so trace=True yields real HW timings."""
    import sys
    import types

    try:
        from antenv.axon_hooks import get_axon_ntff_profile_hook  # noqa: F401
        return
    except ImportError:
        pass
    from trn_agent_boot.trn_boot import _ntff_profile_via_ctypes

    hook = _ntff_profile_via_ctypes("/opt/axon/libaxon_pjrt.so")
    mod = types.ModuleType("antenv.axon_hooks")
    mod.get_axon_ntff_profile_hook = lambda: hook
    mod.set_axon_ntff_profile_hook = lambda h: None
    sys.modules["antenv.axon_hooks"] = mod


def _prep_features(shard):
    x = shard.astype(ml_dtypes.float8_e4m3)  # saturating cast, matches TRN fp8e4
    return np.ascontiguousarray(x.reshape(NB, 128, KC, 128).transpose(3, 0, 2, 1))


def run(inputs, trace=False):
    if trace:
        _ensure_ntff_hook()
    feats = np.asarray(inputs["features"], dtype=np.float32)
    protos = np.asarray(inputs["prototypes"], dtype=np.float32)
    dscale = np.ascontiguousarray(np.asarray(inputs["distance_scale"], dtype=np.float32))
    protos_p = np.zeros((CP, D), dtype=np.float32)
    protos_p[:C] = protos
    nc = build()
    in_maps = [
        {
            "features": _prep_features(feats[i * BS : (i + 1) * BS]),
            "prototypes": protos_p,
            "distance_scale": dscale,
        }
        for i in range(N_CORES)
    ]
    res = run_bass_kernel_spmd(nc, in_maps, core_ids=list(range(N_CORES)),
                               trace=trace)
    out = np.concatenate([r["out"] for r in res.results], axis=0)
    return out, res


def kernel(**inputs) -> np.ndarray:
    out, _ = run(inputs, trace=False)
    return out


# revision 12
# speedup vs baseline: 1.0474x; 1.0474x over previous
"""IsoMaxPlus first-part kernel for TRN2 (8 NeuronCores, data-parallel on B).

out[b, c] = -|s| * sqrt(max(2 - 2 * <f_b/||f_b||, p_c/||p_c||>, 1e-12))

Host prep (per core shard of 8192 rows): features are cast to bf16 and
pre-transposed to d-major layout [128 dpart, 64 blocks, 512] so the device
needs no DMA transposes or casts; prototypes are zero-padded to [1024, 512].

Device per core:
  prolog: 8 big DMAs load all features into SBUF (64KB/partition);
          prototypes are row-normalized (fp32, negated scale, bf16 cast)
          then transposed 128x128-wise on the TensorEngine into
          pnT [128, 4, 1024] bf16.
  main:   8 groups x 8 blocks of 128 rows. Per block: DVE squares the
          bf16 feature slab, PE reduces it against a ones-vector into
          n2 (row norms, exact fp32 psum accumulation), DVE copies n2
          out; per group one ACT Sqrt + DVE reciprocal turn 8 norms into
          the fused scale 2s^2/||f||. Then per block 8 bf16 matmuls
          accumulate dots=-f.p_hat into psum [128,1000], ACT computes
          sqrt(scale*dots + 2s^2) = |s|*dist, GpSimd negates, SP DMAs out.
All matmuls run back-to-back so the PE stays HAM-warm (2.4 GHz).
"""

import numpy as np
from contextlib import ExitStack

import ml_dtypes

import concourse.bass as bass
import concourse.tile as tile
from concourse import bacc, masks, mybir
from concourse.bass import ts
from concourse.bass_utils import run_bass_kernel_spmd

N_CORES = 8
B, D, C = 65536, 512, 1000
CP = 1024                  # prototypes padded (zeros) for 128-alignment
BS = B // N_CORES          # 8192 rows per core
NB = BS // 128             # 64 row blocks
KC = D // 128              # 4 contraction chunks
GB = 8                     # blocks per norm group
NSPLIT = (512, C - 512)    # psum halves (max moving free dim = 512)
CSPLIT = ((0, 256), (256, 256), (512, 256), (768, C - 768))  # fp8 DR chunks
F32 = mybir.dt.float32
BF16 = mybir.dt.bfloat16
F8 = mybir.dt.float8e4
SQRT = mybir.ActivationFunctionType.Sqrt
SQUARE = mybir.ActivationFunctionType.Square


def _emit(nc):
    f_dram = nc.dram_tensor("features", [128, NB, KC, 128], F8, kind="ExternalInput").ap()
    p_dram = nc.dram_tensor("prototypes", [CP, D], F32, kind="ExternalInput").ap()
    s_dram = nc.dram_tensor("distance_scale", [1], F32, kind="ExternalInput").ap()
    o_dram = nc.dram_tensor("out", [BS, C], F32, kind="ExternalOutput").ap()
    o_pair = o_dram.rearrange("(n j p) c -> p n j c", n=NB // 2, j=2, p=128)

    with tile.TileContext(nc) as tc, ExitStack() as ctx:
        singles = ctx.enter_context(tc.tile_pool(name="singles", bufs=1))

        fT = singles.tile([128, NB, KC, 128], F8)  # all features (fp8), 32KB/part
        pnT = singles.tile([128, KC, CP], BF16)    # -p_hat transposed
        identity = singles.tile([128, 128], BF16)
        ones1 = singles.tile([128, 1], BF16)
        n2a = singles.tile([128, NB], F32)         # row norms^2
        scal = singles.tile([128, NB], F32)        # 2s^2 / ||f||
        s_b = singles.tile([128, 1], F32)
        two_s2 = singles.tile([128, 1], F32)
        inv4s4 = singles.tile([128, 1], F32)

        nc.gpsimd.dma_start(out=s_b[:], in_=s_dram.to_broadcast([128, 1]))
        nc.gpsimd.memset(ones1[:], 1.0)
        masks.make_identity(nc, identity[:])
        s2t = singles.tile([128, 1], F32)
        nc.vector.tensor_mul(s2t[:], s_b[:], s_b[:])
        nc.vector.tensor_scalar_mul(two_s2[:], s2t[:], 2.0)
        q4t = singles.tile([128, 1], F32)
        nc.vector.tensor_mul(q4t[:], two_s2[:], two_s2[:])
        nc.vector.reciprocal(inv4s4[:], q4t[:])

        # First two feature groups in flight before the prototype loads so
        # group-0/1 norm phases are not DMA-gated.
        for gi in range(2):
            nc.sync.dma_start(out=fT[:, ts(gi, GB)], in_=f_dram[:, ts(gi, GB)])

        ADD = mybir.AluOpType.add
        npsum = ctx.enter_context(tc.tile_pool(name="npsum", bufs=2, space="PSUM"))
        fsqp = ctx.enter_context(tc.tile_pool(name="fsqp", bufs=2))
        fs2p = ctx.enter_context(tc.tile_pool(name="fs2p", bufs=3))
        fsump = ctx.enter_context(tc.tile_pool(name="fsump", bufs=3))
        gsml = ctx.enter_context(tc.tile_pool(name="gsml", bufs=2))
        opool = ctx.enter_context(tc.tile_pool(name="opool", bufs=4))

        fsq_tiles = {}

        def emit_norm_half(g, half):
            # Row norms for half a group (emitted a group ahead, interleaved
            # between dots half-phases so ACT squares never monopolize the
            # queue while dots-psum sqrt drains are pending). Squares go in
            # 2-block slabs (amortize the 352-cycle ACT fixed cost).
            if half == 0:
                fsq_tiles[g] = fsqp.tile([128, GB, KC, 128], BF16, tag="fsq",
                                         name="fsq")
            fsq = fsq_tiles[g]
            for j0 in range(half * (GB // 2), (half + 1) * (GB // 2), 2):
                nc.scalar.activation(
                    fsq[:, j0 : j0 + 2], fT[:, g * GB + j0 : g * GB + j0 + 2],
                    SQUARE,
                )
            for j in range(half * (GB // 2), (half + 1) * (GB // 2)):
                ib = g * GB + j
                # pairwise-sum the 4 k-chunks on DVE so the PE does one
                # ones-matmul instead of four (bf16 partials: ~0.03% n2 err)
                fs2 = fs2p.tile([128, 2, 128], BF16, tag="fs2")
                nc.vector.tensor_tensor(
                    fs2[:], fsq[:, j, 0:2, :], fsq[:, j, 2:4, :], op=ADD,
                )
                fsum = fsump.tile([128, 128], BF16, tag="fsum")
                nc.vector.tensor_tensor(
                    fsum[:], fs2[:, 0, :], fs2[:, 1, :], op=ADD,
                )
                n2p = npsum.tile([128, 1], F32, tag="n2p")
                nc.tensor.matmul(n2p[:], fsum[:], ones1[:],
                                 skip_group_check=True)
                nc.vector.tensor_copy(n2a[:, ib : ib + 1], n2p[:])
            if half == 1:
                del fsq_tiles[g]
                qg = gsml.tile([128, GB], F32, tag="qg")
                nc.scalar.activation(qg[:], n2a[:, ts(g, GB)], SQRT,
                                     scale=inv4s4[:])
                nc.vector.reciprocal(scal[:, ts(g, GB)], qg[:])

        def emit_norm_phase(g):
            emit_norm_half(g, 0)
            emit_norm_half(g, 1)

        # Norm phases for groups 0/1 ahead of the prototype chain: the PE
        # starts on their ones-matmuls while prototypes are still loading.
        emit_norm_phase(0)
        emit_norm_phase(1)

        # ---- prototypes: normalize rows (negated), TensorE-transpose ----
        with tc.tile_pool(name="tpsum", bufs=2, space="PSUM") as tpsum, \
             tc.tile_pool(name="ppool", bufs=2) as ppool, \
             tc.tile_pool(name="psml", bufs=2) as psml:
            for cb in range(CP // 128):
                pt = ppool.tile([128, D], F32, tag="pt")
                nc.sync.dma_start(out=pt[:], in_=p_dram[ts(cb, 128), :])
                pn2 = psml.tile([128, 1], F32, tag="pn2")
                psq = ppool.tile([128, D], F32, tag="psq")
                nc.scalar.activation(psq[:], pt[:], SQUARE, accum_out=pn2[:])
                nc.scalar.sqrt(pn2[:], pn2[:])
                nc.vector.tensor_scalar_max(pn2[:], pn2[:], 1e-12)
                npri = psml.tile([128, 1], F32, tag="npri")
                nc.vector.reciprocal(npri[:], pn2[:])
                nc.vector.tensor_scalar_mul(npri[:], npri[:], -1.0)
                pnb = ppool.tile([128, D], BF16, tag="pnb")
                nc.vector.tensor_scalar_mul(pnb[:], pt[:], npri[:])
                for kc in range(KC):
                    pst = tpsum.tile([128, 128], BF16, tag="pst")
                    nc.tensor.transpose(pst[:], pnb[:, ts(kc, 128)], identity[:])
                    nc.vector.tensor_copy(pnT[:, kc, ts(cb, 128)], pst[:])

        for gi in range(2, NB // GB):
            nc.sync.dma_start(out=fT[:, ts(gi, GB)], in_=f_dram[:, ts(gi, GB)])

        mpsum = ctx.enter_context(tc.tile_pool(name="mpsum", bufs=3, space="PSUM"))

        def emit_dots_half(g, half):
            for j in range(half * (GB // 2), (half + 1) * (GB // 2), 2):
                ot2 = opool.tile([128, 2, C], F32, tag="ot2")
                for h in range(2):
                    ib = g * GB + j + h
                    dots = mpsum.tile([128, C], F32, tag="dots")
                    for kc in range(KC):
                        for lo, width in ((0, NSPLIT[0]), (NSPLIT[0], NSPLIT[1])):
                            nc.tensor.matmul(
                                dots[:, lo : lo + width],
                                fT[:, ib, kc, :],
                                pnT[:, kc, lo : lo + width],
                                start=(kc == 0), stop=(kc == KC - 1),
                                skip_group_check=True,
                            )
                    nc.scalar.activation(
                        ot2[:, h], dots[:], SQRT,
                        bias=two_s2[:], scale=scal[:, ib : ib + 1],
                    )
                # negate + store two blocks at a time, alternating the two
                # DGE paths (SP HWDGE / GpSimd SWDGE) so neither serializes
                nc.vector.tensor_scalar_mul(ot2[:], ot2[:], -1.0)
                pair = (g * GB + j) // 2
                eng = nc.sync if pair % 2 == 0 else nc.gpsimd
                eng.dma_start(out=o_pair[:, pair], in_=ot2[:])

        for g in range(NB // GB):
            emit_dots_half(g, 0)
            if g + 2 < NB // GB:
                emit_norm_half(g + 2, 0)
            emit_dots_half(g, 1)
            if g + 2 < NB // GB:
                emit_norm_half(g + 2, 1)


def build():
    nc = bacc.Bacc("TRN2", target_bir_lowering=False, debug=False,
                   num_devices=N_CORES)
    _emit(nc)
    nc.compile()
    return nc


def _ensure_ntff_hook():
    """Dev-only: restore the axon NTFF profile hook that the trimmed agent
    image's antenv package lacks, so trace=True yields real HW timings."""
    import sys
    import types

    try:
        from antenv.axon_hooks import get_axon_ntff_profile_hook  # noqa: F401
        return
    except ImportError:
        pass
    from trn_agent_boot.trn_boot import _ntff_profile_via_ctypes

    hook = _ntff_profile_via_ctypes("/opt/axon/libaxon_pjrt.so")
    mod = types.ModuleType("antenv.axon_hooks")
    mod.get_axon_ntff_profile_hook = lambda: hook
    mod.set_axon_ntff_profile_hook = lambda h: None
    sys.modules["antenv.axon_hooks"] = mod


def _prep_features(shard):
    x = shard.astype(ml_dtypes.float8_e4m3)  # saturating cast, matches TRN fp8e4
    return np.ascontiguousarray(x.reshape(NB, 128, KC, 128).transpose(3, 0, 2, 1))


def run(inputs, trace=False):
    if trace:
        _ensure_ntff_hook()
    feats = np.asarray(inputs["features"], dtype=np.float32)
    protos = np.asarray(inputs["prototypes"], dtype=np.float32)
    dscale = np.ascontiguousarray(np.asarray(inputs["distance_scale"], dtype=np.float32))
    protos_p = np.zeros((CP, D), dtype=np.float32)
    protos_p[:C] = protos
    nc = build()
    in_maps = [
        {
            "features": _prep_features(feats[i * BS : (i + 1) * BS]),
            "prototypes": protos_p,
            "distance_scale": dscale,
        }
        for i in range(N_CORES)
    ]
    res = run_bass_kernel_spmd(nc, in_maps, core_ids=list(range(N_CORES)),
                               trace=trace)
    out = np.concatenate([r["out"] for r in res.results], axis=0)
    return out, res


def kernel(**inputs) -> np.ndarray:
    out, _ = run(inputs, trace=False)
    return out
